# revision 12
# baseline (speedup 1.0000x reference)
"""Multi-head causal attention with RoPE on 8 TRN2 NeuronCores.

Sharding: 8 cores = 2 batches x 4 head-groups (4 heads each).
Per-core Bass kernel computes the group-partial output transposed in
fp16; host sums group partials and transposes back.

v2 design (vs fp32r baseline):
- fp16 operands for every matmul (1 cycle/row like bf16, 4x cheaper
  LDWEIGHTS than fp32r, ~4x better mantissa than bf16).
- Single fused loop over query slices: projections(u), attention(u)
  and output-projection(u) share one pool scope so the Tile scheduler
  overlaps them; no phase barrier, no DRAM roundtrip for Q/K/V.
- Softmax denominator via DVE accumulation of exp tiles (fp16 fast
  mode) + one ones-matmul per (head, slice) instead of a ones-matmul
  per score tile (kills 144 PE matmuls per core).
- Causal mask is preloaded into PSUM by GpSimd and the scores matmul
  accumulates on top (start=False), so masking is off the
  scores->exp critical path.
- PSUM drains spread across ACT (Q/K RoPE staging) and GpSimd
  (V, mask preload, Wo drain); DVE does RoPE/accum/normalize in
  2-byte fast mode.
- Host pre-arranges all DRAM layouts so each weight loads in one DMA.
"""

import numpy as np

import concourse.bass as bass  # noqa: F401
import concourse.tile as tile
from concourse import bacc, mybir

B, S, D, H, HD = 2, 2048, 2048, 16, 128
NCORES = 8
G = 4            # head groups
GH = 4           # heads per group
GD = GH * HD     # 512 dims per group
P = 128
SQ_U = S // 512  # 4 query slices
SK_T = S // P    # 16 key tiles
MASK_NEG = -30000.0

_f32 = mybir.dt.float32
_f16 = mybir.dt.float16

_cache = {}


def _build(causal: bool, reps: int = 1, depth: int = 4):
    nc = bacc.Bacc("TRN2", target_bir_lowering=False, debug=False)
    # all host-prearranged [128, ...] layouts, fp16
    xh = nc.dram_tensor("xh", [P, SK_T, S], _f16, kind="ExternalInput").ap()
    wq = nc.dram_tensor("wq", [P, SK_T * GD], _f16, kind="ExternalInput").ap()
    wk = nc.dram_tensor("wk", [P, SK_T * GD], _f16, kind="ExternalInput").ap()
    wv = nc.dram_tensor("wv", [P, SK_T * GD], _f16, kind="ExternalInput").ap()
    wo = nc.dram_tensor("wo", [P, GH * S], _f16, kind="ExternalInput").ap()
    csq = nc.dram_tensor("csq", [P, S], _f16, kind="ExternalInput").ap()
    ssq = nc.dram_tensor("ssq", [P, S], _f16, kind="ExternalInput").ap()
    csk = nc.dram_tensor("csk", [P, S], _f16, kind="ExternalInput").ap()
    ssk = nc.dram_tensor("ssk", [P, S], _f16, kind="ExternalInput").ap()
    if causal:
        maskd = nc.dram_tensor("maskd", [P, 4 * 512], _f16, kind="ExternalInput").ap()
    else:
        maskd = nc.dram_tensor("maskd", [P, SQ_U, SK_T * 512], _f16,
                               kind="ExternalInput").ap()
    outT = nc.dram_tensor("outT", [P, SK_T, S], _f16, kind="ExternalOutput").ap()

    with tile.TileContext(nc) as tc:
        with (
            tc.tile_pool(name="persist", bufs=1) as persist,
            tc.tile_pool(name="px", bufs=4) as px,
            tc.tile_pool(name="ptmp", bufs=4) as ptmp,
            tc.tile_pool(name="app", bufs=8) as app,
            tc.tile_pool(name="pacc", bufs=3) as paccp,
            tc.tile_pool(name="par", bufs=2) as par,
            tc.tile_pool(name="pwst", bufs=4) as pwst,
            tc.tile_pool(name="pmu", bufs=2) as pmu,
            tc.tile_pool(name="ps_proj", bufs=2, space="PSUM") as ps_proj,
            tc.tile_pool(name="ps_s", bufs=3, space="PSUM") as ps_s,
            tc.tile_pool(name="ps_a", bufs=2, space="PSUM") as ps_a,
            tc.tile_pool(name="ps_o", bufs=1, space="PSUM") as ps_o,
        ):
          for _rep in range(reps):
            # --- persistent SBUF tiles + one-DMA loads, ordered so the
            # first Q chain can start ASAP
            wq_s = persist.tile([P, SK_T * GD], _f16, tag="wq")
            wk_s = persist.tile([P, SK_T * GD], _f16, tag="wk")
            wv_s = persist.tile([P, SK_T * GD], _f16, tag="wv")
            wo_s = persist.tile([P, GH * S], _f16, tag="wo")
            csq_s = persist.tile([P, S], _f16, tag="csq")
            ssq_s = persist.tile([P, S], _f16, tag="ssq")
            csk_s = persist.tile([P, S], _f16, tag="csk")
            ssk_s = persist.tile([P, S], _f16, tag="ssk")
            ones_s = persist.tile([P, P], _f16, tag="ones")
            qt_s = persist.tile([P, GH * S], _f16, tag="qt")
            kt_s = persist.tile([P, GH * S], _f16, tag="kt")
            v_s = persist.tile([P, SK_T * GD], _f16, tag="v")
            aot_s = persist.tile([P, GH * S], _f16, tag="aot")
            if causal:
                md_s = persist.tile([P, 4 * 512], _f16, tag="md")

            for c in range(4):
                cw = SK_T * GD // 4
                nc.sync.dma_start(wq_s[:, c * cw:(c + 1) * cw],
                                  wq[:, c * cw:(c + 1) * cw])

            def _load_xu(u, chunks=1):
                xhalves = []
                for half in range(2):
                    xu = px.tile([P, 8, 512], _f16, tag="xu")
                    cn = 8 // chunks
                    for c in range(chunks):
                        nc.sync.dma_start(
                            xu[:, c * cn:(c + 1) * cn, :],
                            xh[:, half * 8 + c * cn: half * 8 + (c + 1) * cn,
                               u * 512:(u + 1) * 512])
                    xhalves.append(xu)
                return xhalves

            xh0 = _load_xu(0, chunks=4)
            nc.sync.dma_start(csq_s[:], csq[:])
            nc.sync.dma_start(ssq_s[:], ssq[:])
            nc.sync.dma_start(wk_s[:], wk[:])
            nc.sync.dma_start(csk_s[:], csk[:])
            nc.sync.dma_start(ssk_s[:], ssk[:])
            nc.sync.dma_start(wv_s[:], wv[:])
            nc.gpsimd.memset(ones_s[:], 1.0)
            if causal:
                nc.sync.dma_start(md_s[:], maskd[:])
            nc.sync.dma_start(wo_s[:], wo[:])

            for u in range(SQ_U):
                xu = xh0 if u == 0 else _load_xu(u)
                su = slice(u * 512, (u + 1) * 512)
                if not causal:
                    mu = pmu.tile([P, SK_T * 512], _f16, tag="mu")
                    nc.sync.dma_start(mu[:], maskd[:, u, :])

                # ---- Q^T / K^T projections with fused RoPE -> SBUF fp16
                for (w_s, dst, cs_s, ss_s) in (
                        (wq_s, qt_s, csq_s, ssq_s), (wk_s, kt_s, csk_s, ssk_s)):
                    for dt in range(GH):
                        pq = ps_proj.tile([P, 512], _f32, tag="pq")
                        for t in range(SK_T):
                            nc.tensor.matmul(
                                pq[:],
                                w_s[:, t * GD + dt * P: t * GD + dt * P + P],
                                xu[t // 8][:, t % 8, :],
                                start=(t == 0), stop=(t == SK_T - 1))
                        # RoPE on DVE straight from PSUM (gpsimd cannot
                        # touch PSUM); final add runs in 2-byte fast mode
                        t1 = ptmp.tile([P, 512], _f16, tag="t1")
                        t2 = ptmp.tile([P, 512], _f16, tag="t2")
                        nc.vector.tensor_mul(t1[:], pq[:], cs_s[:, su])
                        nc.vector.tensor_mul(t2[0:64, :], pq[64:P, :], ss_s[0:64, su])
                        nc.vector.tensor_mul(t2[64:P, :], pq[0:64, :], ss_s[64:P, su])
                        nc.vector.tensor_add(
                            dst[:, dt * S + u * 512: dt * S + (u + 1) * 512],
                            t1[:], t2[:])

                # ---- V projection (natural layout) -> SBUF fp16
                for st in range(4):
                    g = 4 * u + st
                    pv = ps_proj.tile([P, GD], _f32, tag="pq")
                    for t in range(SK_T):
                        nc.tensor.matmul(
                            pv[:],
                            xu[t // 8][:, t % 8, st * P:(st + 1) * P],
                            wv_s[:, t * GD:(t + 1) * GD],
                            start=(t == 0), stop=(t == SK_T - 1))
                    nc.scalar.copy(v_s[:, g * GD:(g + 1) * GD], pv[:])

                # ---- attention for this query slice
                n_sk = 4 * (u + 1) if causal else SK_T
                for h in range(GH):
                    qu_base = h * S + u * 512
                    psa = ps_a.tile([P, 512], _f32, tag="a")
                    acc = paccp.tile([P, 512], _f16, tag="acc")
                    pts = [None] * n_sk
                    los = [0] * n_sk   # first live query column per tile

                    def _consume(t):
                        lo = los[t]
                        nc.tensor.matmul(
                            psa[:, lo:512],
                            v_s[:, t * GD + h * P: t * GD + h * P + P],
                            pts[t][:, lo:512],
                            start=(t == 0), stop=(t == n_sk - 1))

                    for t in range(n_sk):
                        pss = ps_s.tile([P, 512], _f32, tag="s")
                        if causal:
                            masked = t >= 4 * u
                            j = t - 4 * u
                            msrc = md_s[:, j * 512:(j + 1) * 512] if masked else None
                            # queries < 128j see no unmasked key in this tile
                            lo = 128 * j if masked else 0
                        else:
                            masked = True
                            msrc = mu[:, t * 512:(t + 1) * 512]
                            lo = 0
                        los[t] = lo
                        sl = slice(lo, 512)
                        # u == 0: start=True + DVE mask add; this also puts
                        # every ps_s bank into a known started-once state so
                        # the u > 0 preload + start=False accumulate is safe
                        # regardless of initial PSUM bank state.
                        preload = masked and u > 0
                        if preload:
                            nc.vector.tensor_copy(pss[:, sl], msrc[:, sl])
                        nc.tensor.matmul(
                            pss[:, sl],
                            kt_s[:, h * S + t * P: h * S + (t + 1) * P],
                            qt_s[:, qu_base + lo: qu_base + 512],
                            start=(not preload), stop=True,
                            skip_group_check=preload)
                        if masked and not preload:
                            nc.vector.tensor_add(pss[:, sl], pss[:, sl], msrc[:, sl])
                        pt = app.tile([P, 512], _f16, tag="p")
                        nc.scalar.activation(pt[:, sl], pss[:, sl],
                                             mybir.ActivationFunctionType.Exp)
                        pts[t] = pt
                        if t == 0:
                            nc.vector.tensor_copy(acc[:], pt[:])
                        else:
                            nc.vector.tensor_add(acc[:, sl], acc[:, sl], pt[:, sl])
                        if t >= depth:
                            _consume(t - depth)
                    for t in range(max(0, n_sk - depth), n_sk):
                        _consume(t)
                    psd = ps_s.tile([P, 512], _f32, tag="s")
                    nc.tensor.matmul(psd[:], ones_s[:], acc[:],
                                     start=True, stop=True)
                    rec = par.tile([P, 512], _f32, tag="rec")
                    nc.vector.reciprocal(rec[:], psd[:])
                    nc.vector.tensor_mul(
                        aot_s[:, h * S + u * 512: h * S + (u + 1) * 512],
                        psa[:], rec[:])

                # ---- output projection for this slice
                for ot in range(SK_T):
                    po = ps_o.tile([P, 512], _f32, tag="o")
                    for dt in range(GH):
                        nc.tensor.matmul(
                            po[:],
                            wo_s[:, dt * S + ot * P: dt * S + (ot + 1) * P],
                            aot_s[:, dt * S + u * 512: dt * S + (u + 1) * 512],
                            start=(dt == 0), stop=(dt == GH - 1))
                    so = pwst.tile([P, 512], _f16, tag="so")
                    nc.vector.tensor_copy(so[:], po[:])
                    nc.sync.dma_start(outT[:, ot, u * 512:(u + 1) * 512], so[:])
    nc.compile()
    return nc


class _Runner:
    """Persistent PJRT executable for one compiled Bass module (SPMD over 8 cores)."""

    def __init__(self, nc, n_cores):
        import jax
        from jax.sharding import Mesh, PartitionSpec
        from jax.experimental.shard_map import shard_map
        from concourse.bass2jax import (
            _bass_exec_p, install_neuronx_cc_hook, partition_id_tensor)

        install_neuronx_cc_hook()
        self.jax = jax
        self.n_cores = n_cores
        partition_name = nc.partition_id_tensor.name if nc.partition_id_tensor else None
        in_names, out_names, out_avals = [], [], []
        for alloc in nc.m.functions[0].allocations:
            if not isinstance(alloc, mybir.MemoryLocationSet):
                continue
            name = alloc.memorylocations[0].name
            if alloc.kind == "ExternalInput":
                if name != partition_name:
                    in_names.append(name)
            elif alloc.kind == "ExternalOutput":
                out_names.append(name)
                out_avals.append(jax.core.ShapedArray(
                    tuple(alloc.tensor_shape), mybir.dt.np(alloc.dtype)))
        self.in_names, self.out_names, self.out_avals = in_names, out_names, out_avals
        n_params, n_outs = len(in_names), len(out_avals)
        all_in = list(in_names) + list(out_names)
        if partition_name is not None:
            all_in.append(partition_name)

        def _body(*args):
            operands = list(args)
            if partition_name is not None:
                operands.append(partition_id_tensor())
            return tuple(_bass_exec_p.bind(
                *operands,
                out_avals=tuple(out_avals), in_names=tuple(all_in),
                out_names=tuple(out_names), lowering_input_output_aliases=(),
                sim_require_finite=True, sim_require_nnan=True, nc=nc))

        devices = jax.devices()[:n_cores]
        mesh = Mesh(np.asarray(devices), ("core",))
        self.sharding = jax.sharding.NamedSharding(mesh, PartitionSpec("core"))
        self.fn = jax.jit(
            shard_map(_body, mesh=mesh,
                      in_specs=(PartitionSpec("core"),) * (n_params + n_outs),
                      out_specs=(PartitionSpec("core"),) * n_outs,
                      check_rep=False),
            keep_unused=True)
        self._dev_args = None

    def put_inputs(self, in_maps):
        jax = self.jax
        concat_in = [
            np.concatenate([np.asarray(in_maps[c][n]) for c in range(self.n_cores)], axis=0)
            for n in self.in_names]
        concat_zeros = [
            np.zeros((self.n_cores * a.shape[0], *a.shape[1:]), a.dtype)
            for a in self.out_avals]
        self._dev_args = [
            jax.device_put(v, self.sharding) for v in concat_in + concat_zeros]
        for a in self._dev_args:
            a.block_until_ready()

    def execute(self):
        return self.fn(*self._dev_args)

    def run(self, in_maps):
        last_err = None
        for attempt in range(3):
            try:
                self.put_inputs(in_maps)
                outs = self.execute()
                self.jax.block_until_ready(outs)
                return [
                    {n: np.asarray(outs[i]).reshape(
                        self.n_cores, *self.out_avals[i].shape)[c]
                     for i, n in enumerate(self.out_names)}
                    for c in range(self.n_cores)]
            except Exception as e:  # transient NRT faults: retry
                last_err = e
                import time
                time.sleep(2.0 * (attempt + 1))
        raise last_err


def _get_runner(causal: bool):
    if causal not in _cache:
        _cache[causal] = _Runner(_build(causal), NCORES)
    return _cache[causal]


def _to_tiled(a, n):
    """[n*128, C] row-major -> [128, n*C] fp16 chunk layout."""
    c = a.shape[1]
    return np.ascontiguousarray(
        a.reshape(n, P, c).transpose(1, 0, 2).reshape(P, n * c).astype(np.float16))


def _host_prep(x, mask, Wq, Wk, Wv, Wo, causal):
    scale = np.float32(1.0) / np.sqrt(np.float32(HD))
    perm = np.concatenate(
        [np.concatenate([np.arange(0, HD, 2), np.arange(1, HD, 2)]) + HD * hh
         for hh in range(GH)])
    inv = (np.float32(1.0) / np.power(
        np.float32(10000.0),
        np.arange(0, HD, 2).astype(np.float32) / np.float32(HD))).astype(np.float32)
    ang = np.arange(S, dtype=np.float32)[:, None] * inv[None, :]
    cos_t = np.cos(ang).T.astype(np.float32)
    sin_t = np.sin(ang).T.astype(np.float32)
    cs = np.ascontiguousarray(np.concatenate([cos_t, cos_t], axis=0))
    ss = np.ascontiguousarray(np.concatenate([-sin_t, sin_t], axis=0))
    csq_h = (cs * scale).astype(np.float16)
    ssq_h = (ss * scale).astype(np.float16)
    csk_h = cs.astype(np.float16)
    ssk_h = ss.astype(np.float16)
    if causal:
        md = np.empty((P, 4 * 512), np.float32)
        for j in range(4):
            kl = np.arange(P)[:, None]
            q = np.arange(512)[None, :]
            md[:, j * 512:(j + 1) * 512] = np.where(
                128 * j + kl <= q, 0.0, MASK_NEG)
        md_h = md.astype(np.float16)
    else:
        maskT = np.clip(mask.T, MASK_NEG, -MASK_NEG).astype(np.float32)
        md = np.empty((P, SQ_U, SK_T * 512), np.float32)
        for u in range(SQ_U):
            for t in range(SK_T):
                md[:, u, t * 512:(t + 1) * 512] = \
                    maskT[t * P:(t + 1) * P, u * 512:(u + 1) * 512]
        md_h = md.astype(np.float16)

    xh_hs = [
        _to_tiled(np.ascontiguousarray(x[b].T), SK_T).reshape(P, SK_T, S)
        for b in range(B)]
    in_maps = []
    for c in range(NCORES):
        b, g = c // G, c % G
        rows = slice(g * GD, (g + 1) * GD)
        m = {
            "xh": xh_hs[b],
            "wq": _to_tiled(np.ascontiguousarray(Wq[rows].T[:, perm]), SK_T),
            "wk": _to_tiled(np.ascontiguousarray(Wk[rows].T[:, perm]), SK_T),
            "wv": _to_tiled(np.ascontiguousarray(Wv[rows].T), SK_T),
            "wo": _to_tiled(np.ascontiguousarray(Wo[:, rows].T), GH),
            "csq": csq_h, "ssq": ssq_h, "csk": csk_h, "ssk": ssk_h,
            "maskd": md_h,
        }
        in_maps.append(m)
    return in_maps


def kernel(x, mask, Wq, Wk, Wv, Wo):
    x = np.asarray(x, dtype=np.float32)
    mask = np.asarray(mask, dtype=np.float32)
    Wq = np.asarray(Wq, dtype=np.float32)
    Wk = np.asarray(Wk, dtype=np.float32)
    Wv = np.asarray(Wv, dtype=np.float32)
    Wo = np.asarray(Wo, dtype=np.float32)
    expected_mask = np.triu(np.full((S, S), -1e9, dtype=np.float32), k=1)
    causal = bool(np.array_equal(mask, expected_mask))
    runner = _get_runner(causal)
    in_maps = _host_prep(x, mask, Wq, Wk, Wv, Wo, causal)
    results = runner.run(in_maps)
    out = np.empty((B, S, D), np.float32)
    for b in range(B):
        acc = results[b * G]["outT"].astype(np.float32)
        for g in range(1, G):
            acc += results[b * G + g]["outT"].astype(np.float32)
        # outT[p, ot, s] = out^T[ot*128+p, s]
        out[b] = acc.transpose(1, 0, 2).reshape(D, S).T
    return out
